# revision 1
# baseline (speedup 1.0000x reference)
"""Trainium2 Bass kernel for nn_ClassificationLayer (Gaussian pdf-sum classifier).

Math:
  mu/sd per dim from tiny [128,10] reference sets (host, exact).
  Per row i: s_n[i] = sum_d INV_SQRT_2PI/sd_d * exp(-0.5*((x[i,d]-mu_d)/sd_d)^2)
  (same for anomaly), then the batch recurrence p_k = (p_{k-1} + s_k)/128,
  output = [pn/(pn+pa), pa/(pn+pa)].

Device strategy (8 cores, data-parallel over N):
  - Host transposes each core's row-shard to [128 dims, R rows] so per-dim
    constants become per-partition scale/bias for the ScalarEngine.
  - One ACTIVATE per distribution per tile: Derivative_Erf(scale*x + bias)
    = (2/sqrt(pi)) * exp(-((x-mu)/sd)^2/2)  -- the whole Gaussian in one pass.
  - Reduction over dims (partitions) via TensorEngine matvec in fp32r
    (1 cycle/row). The stationary operand is a shifted window over a
    zero-padded weight buffer so chunk g's sums land in PSUM partition g;
    123 matmuls accumulate into one PSUM bank per distribution.
  - The scalar recurrence decays by 1/128 per step (a^13 ~ 1e-28), so it is
    re-run exactly on the gathered per-row sums on host as a short causal
    convolution in float64.
"""

import numpy as np

N, DIM, S = 500000, 128, 10
INV_SQRT_2PI = 0.3989422804014327
NCORES = 8
CHUNK = 512                      # rows per matvec (PSUM bank free-dim)
NCHUNK = 123                     # chunks per core  (123*512 = 62976 rows)
R = NCHUNK * CHUNK               # rows per core, 8*R = 503808 >= N
TILE_W = 4096                    # rows per ACT tile
# 15 full tiles + one 1536 tail = 62976
TILES = [(i * TILE_W, TILE_W) for i in range(15)] + [(15 * TILE_W, R - 15 * TILE_W)]

_COMPILED = None
LAST_RESULTS = None  # BassKernelResults of the most recent device run


def _build():
    import concourse.tile as tile
    from concourse import bacc, mybir

    nc = bacc.Bacc("TRN2", target_bir_lowering=False, debug=False,
                   num_devices=NCORES)

    xT = nc.dram_tensor("xT", [DIM, R], mybir.dt.float32,
                        kind="ExternalInput").ap()
    # consts: col 0 scale_n, 1 bias_n, 2 scale_a, 3 bias_a
    consts = nc.dram_tensor("consts", [DIM, 4], mybir.dt.float32,
                            kind="ExternalInput").ap()
    # wmat: cols [0,256) normal weights (c_n at col 128), cols [256,512)
    # anomaly weights (c_a at col 384); everything else exactly 0.
    wmat = nc.dram_tensor("wmat", [DIM, 512], mybir.dt.float32r,
                          kind="ExternalInput").ap()
    sn_out = nc.dram_tensor("sn_out", [128, CHUNK], mybir.dt.float32,
                            kind="ExternalOutput").ap()
    sa_out = nc.dram_tensor("sa_out", [128, CHUNK], mybir.dt.float32,
                            kind="ExternalOutput").ap()

    DErf = mybir.ActivationFunctionType.Derivative_Erf

    with tile.TileContext(nc) as tc:
        with tc.tile_pool(name="cpool", bufs=1) as cpool, \
             tc.tile_pool(name="xpool", bufs=3) as xpool, \
             tc.tile_pool(name="epool", bufs=2) as epool, \
             tc.tile_pool(name="pspool", bufs=1, space="PSUM") as pspool:

            consts_t = cpool.tile([DIM, 4], mybir.dt.float32)
            nc.sync.dma_start(consts_t[:], consts[:, :])
            w_t = cpool.tile([DIM, 512], mybir.dt.float32r)
            nc.sync.dma_start(w_t[:], wmat[:, :])

            sn_ps = pspool.tile([128, CHUNK], mybir.dt.float32)
            sa_ps = pspool.tile([128, CHUNK], mybir.dt.float32)

            g = 0
            for (off, w) in TILES:
                x_t = xpool.tile([DIM, w], mybir.dt.float32, tag="x",
                                 padded_shape=[DIM, TILE_W])
                nc.sync.dma_start(x_t[:], xT[:, off:off + w])
                en_t = epool.tile([DIM, w], mybir.dt.float32r, tag="en",
                                  padded_shape=[DIM, TILE_W])
                nc.scalar.activation(en_t[:], x_t[:], DErf,
                                     bias=consts_t[:, 1:2],
                                     scale=consts_t[:, 0:1])
                ea_t = epool.tile([DIM, w], mybir.dt.float32r, tag="ea",
                                  padded_shape=[DIM, TILE_W])
                nc.scalar.activation(ea_t[:], x_t[:], DErf,
                                     bias=consts_t[:, 3:4],
                                     scale=consts_t[:, 2:3])
                for c in range(w // CHUNK):
                    sl = slice(c * CHUNK, (c + 1) * CHUNK)
                    nc.tensor.matmul(sn_ps[:], w_t[:, 128 - g:256 - g],
                                     en_t[:, sl], start=(g == 0),
                                     stop=(g == NCHUNK - 1),
                                     skip_group_check=True)
                    nc.tensor.matmul(sa_ps[:], w_t[:, 384 - g:512 - g],
                                     ea_t[:, sl], start=(g == 0),
                                     stop=(g == NCHUNK - 1),
                                     skip_group_check=True)
                    g += 1

            sn_sb = cpool.tile([128, CHUNK], mybir.dt.float32)
            nc.vector.tensor_copy(sn_sb[:], sn_ps[:])
            sa_sb = cpool.tile([128, CHUNK], mybir.dt.float32)
            nc.vector.tensor_copy(sa_sb[:], sa_ps[:])
            nc.sync.dma_start(sn_out[:, :], sn_sb[:])
            nc.sync.dma_start(sa_out[:, :], sa_sb[:])

    nc.compile()
    return nc


def _get_compiled():
    global _COMPILED
    if _COMPILED is None:
        _COMPILED = _build()
    return _COMPILED


def kernel(encoded, normal_dist, anomaly_dist):
    global LAST_RESULTS
    from concourse.bass_utils import run_bass_kernel_spmd

    x = np.ascontiguousarray(np.asarray(encoded, dtype=np.float32))
    nd = np.asarray(normal_dist, dtype=np.float64)
    ad = np.asarray(anomaly_dist, dtype=np.float64)

    # per-dim stats (torch defaults: unbiased std)
    mu_n = nd.mean(axis=1)
    sd_n = nd.std(axis=1, ddof=1)
    mu_a = ad.mean(axis=1)
    sd_a = ad.std(axis=1, ddof=1)
    isd_n, isd_a = 1.0 / sd_n, 1.0 / sd_a

    inv_sqrt2 = 1.0 / np.sqrt(2.0)
    consts = np.stack([
        isd_n * inv_sqrt2,            # scale_n
        -mu_n * isd_n * inv_sqrt2,    # bias_n
        isd_a * inv_sqrt2,            # scale_a
        -mu_a * isd_a * inv_sqrt2,    # bias_a
    ], axis=1).astype(np.float32)     # [128, 4]

    half_sqrt_pi = 0.5 * np.sqrt(np.pi)
    c_n = (INV_SQRT_2PI * isd_n * half_sqrt_pi).astype(np.float32)
    c_a = (INV_SQRT_2PI * isd_a * half_sqrt_pi).astype(np.float32)
    wmat = np.zeros((DIM, 512), dtype=np.float32)
    wmat[:, 128] = c_n
    wmat[:, 384] = c_a

    in_maps = []
    for i in range(NCORES):
        lo = i * R
        hi = min(lo + R, N)
        shard_T = np.zeros((DIM, R), dtype=np.float32)
        shard_T[:, :hi - lo] = x[lo:hi].T
        in_maps.append({"xT": shard_T, "consts": consts, "wmat": wmat})

    nc = _get_compiled()
    res = run_bass_kernel_spmd(nc, in_maps, core_ids=list(range(NCORES)))
    LAST_RESULTS = res

    s_n = np.empty(N, dtype=np.float64)
    s_a = np.empty(N, dtype=np.float64)
    for i in range(NCORES):
        lo = i * R
        hi = min(lo + R, N)
        s_n[lo:hi] = res.results[i]["sn_out"].reshape(-1)[:hi - lo]
        s_a[lo:hi] = res.results[i]["sa_out"].reshape(-1)[:hi - lo]

    # exact recurrence p_k = (p_{k-1} + s_k)/dim as truncated causal
    # convolution: p_k = sum_j (1/dim)^(j+1) s_{k-j}; (1/128)^14 ~ 3e-30.
    a = 1.0 / DIM
    pn = np.zeros(N, dtype=np.float64)
    pa = np.zeros(N, dtype=np.float64)
    wgt = a
    for j in range(14):
        if j == 0:
            pn += wgt * s_n
            pa += wgt * s_a
        else:
            pn[j:] += wgt * s_n[:-j]
            pa[j:] += wgt * s_a[:-j]
        wgt *= a
    total = pn + pa
    out = np.empty((N, 2), dtype=np.float32)
    out[:, 0] = (pn / total).astype(np.float32)
    out[:, 1] = (pa / total).astype(np.float32)
    return out


# revision 3
# speedup vs baseline: 1.0228x; 1.0228x over previous
"""Trainium2 Bass kernel for nn_ClassificationLayer (Gaussian pdf-sum classifier).

Math:
  mu/sd per dim from tiny [128,10] reference sets (host, exact).
  Per row i: s_n[i] = sum_d INV_SQRT_2PI/sd_d * exp(-0.5*((x[i,d]-mu_d)/sd_d)^2)
  (same for anomaly), then the batch recurrence p_k = (p_{k-1} + s_k)/128,
  output = [pn/(pn+pa), pa/(pn+pa)].

Device strategy (8 cores, data-parallel over N):
  - Host transposes each core's row-shard to [128 dims, R rows] so per-dim
    constants become per-partition scale/bias for the ScalarEngine.
  - One ACTIVATE per distribution per tile: Derivative_Erf(scale*x + bias)
    = (2/sqrt(pi)) * exp(-((x-mu)/sd)^2/2)  -- the whole Gaussian in one pass.
  - Reduction over dims (partitions) via TensorEngine matvec in fp32r
    (1 cycle/row). The stationary operand is a shifted window over a
    zero-padded weight buffer so chunk g's sums land in PSUM partition g;
    123 matmuls accumulate into one PSUM bank per distribution.
  - The scalar recurrence decays by 1/128 per step (a^13 ~ 1e-28), so it is
    re-run exactly on the gathered per-row sums on host as a short causal
    convolution in float64.
"""

import numpy as np

N, DIM, S = 500000, 128, 10
INV_SQRT_2PI = 0.3989422804014327
NCORES = 8
CHUNK = 512                      # rows per matvec (PSUM bank free-dim)
NCHUNK = 123                     # chunks per core  (123*512 = 62976 rows)
R = NCHUNK * CHUNK               # rows per core, 8*R = 503808 >= N
# ACT tile widths: small head tiles so the ScalarEngine starts before the
# first big DMA lands, then wide tiles to amortize the 224-cycle ScalarE
# per-instruction overhead; 512 tail drains fast. Sum = 62976 = R.
TILE_WS = [1024, 2048, 4096] + [6144] * 9 + [512]
assert sum(TILE_WS) == R
_off = 0
TILES = []
for _w in TILE_WS:
    TILES.append((_off, _w))
    _off += _w
MAX_W = max(TILE_WS)

_COMPILED = None
LAST_RESULTS = None  # BassKernelResults of the most recent device run


def _build():
    import concourse.tile as tile
    from concourse import bacc, mybir

    nc = bacc.Bacc("TRN2", target_bir_lowering=False, debug=False,
                   num_devices=NCORES)

    xT = nc.dram_tensor("xT", [DIM, R], mybir.dt.float32,
                        kind="ExternalInput").ap()
    # consts: col 0 scale_n, 1 bias_n, 2 scale_a, 3 bias_a
    consts = nc.dram_tensor("consts", [DIM, 4], mybir.dt.float32,
                            kind="ExternalInput").ap()
    # wmat: cols [0,256) normal weights (c_n at col 128), cols [256,512)
    # anomaly weights (c_a at col 384); everything else exactly 0.
    wmat = nc.dram_tensor("wmat", [DIM, 512], mybir.dt.float32r,
                          kind="ExternalInput").ap()
    sn_out = nc.dram_tensor("sn_out", [128, CHUNK], mybir.dt.float32,
                            kind="ExternalOutput").ap()
    sa_out = nc.dram_tensor("sa_out", [128, CHUNK], mybir.dt.float32,
                            kind="ExternalOutput").ap()

    DErf = mybir.ActivationFunctionType.Derivative_Erf

    with tile.TileContext(nc) as tc:
        with tc.tile_pool(name="cpool", bufs=1) as cpool, \
             tc.tile_pool(name="xpool", bufs=3) as xpool, \
             tc.tile_pool(name="epool", bufs=2) as epool, \
             tc.tile_pool(name="pspool", bufs=1, space="PSUM") as pspool:

            consts_t = cpool.tile([DIM, 4], mybir.dt.float32)
            nc.sync.dma_start(consts_t[:], consts[:, :])
            w_t = cpool.tile([DIM, 512], mybir.dt.float32r)
            nc.sync.dma_start(w_t[:], wmat[:, :])

            sn_ps = pspool.tile([128, CHUNK], mybir.dt.float32)
            sa_ps = pspool.tile([128, CHUNK], mybir.dt.float32)

            g = 0
            for (off, w) in TILES:
                x_t = xpool.tile([DIM, w], mybir.dt.float32, tag="x",
                                 padded_shape=[DIM, MAX_W])
                nc.sync.dma_start(x_t[:], xT[:, off:off + w])
                en_t = epool.tile([DIM, w], mybir.dt.float32r, tag="en",
                                  padded_shape=[DIM, MAX_W])
                nc.scalar.activation(en_t[:], x_t[:], DErf,
                                     bias=consts_t[:, 1:2],
                                     scale=consts_t[:, 0:1])
                ea_t = epool.tile([DIM, w], mybir.dt.float32r, tag="ea",
                                  padded_shape=[DIM, MAX_W])
                nc.scalar.activation(ea_t[:], x_t[:], DErf,
                                     bias=consts_t[:, 3:4],
                                     scale=consts_t[:, 2:3])
                for c in range(w // CHUNK):
                    sl = slice(c * CHUNK, (c + 1) * CHUNK)
                    nc.tensor.matmul(sn_ps[:], w_t[:, 128 - g:256 - g],
                                     en_t[:, sl], start=(g == 0),
                                     stop=(g == NCHUNK - 1),
                                     skip_group_check=True)
                    nc.tensor.matmul(sa_ps[:], w_t[:, 384 - g:512 - g],
                                     ea_t[:, sl], start=(g == 0),
                                     stop=(g == NCHUNK - 1),
                                     skip_group_check=True)
                    g += 1

            sn_sb = cpool.tile([128, CHUNK], mybir.dt.float32)
            nc.vector.tensor_copy(sn_sb[:], sn_ps[:])
            sa_sb = cpool.tile([128, CHUNK], mybir.dt.float32)
            nc.vector.tensor_copy(sa_sb[:], sa_ps[:])
            nc.sync.dma_start(sn_out[:, :], sn_sb[:])
            nc.sync.dma_start(sa_out[:, :], sa_sb[:])

    nc.compile()
    return nc


def _get_compiled():
    global _COMPILED
    if _COMPILED is None:
        _COMPILED = _build()
    return _COMPILED


def kernel(encoded, normal_dist, anomaly_dist):
    global LAST_RESULTS
    from concourse.bass_utils import run_bass_kernel_spmd

    x = np.ascontiguousarray(np.asarray(encoded, dtype=np.float32))
    nd = np.asarray(normal_dist, dtype=np.float64)
    ad = np.asarray(anomaly_dist, dtype=np.float64)

    # per-dim stats (torch defaults: unbiased std)
    mu_n = nd.mean(axis=1)
    sd_n = nd.std(axis=1, ddof=1)
    mu_a = ad.mean(axis=1)
    sd_a = ad.std(axis=1, ddof=1)
    isd_n, isd_a = 1.0 / sd_n, 1.0 / sd_a

    inv_sqrt2 = 1.0 / np.sqrt(2.0)
    consts = np.stack([
        isd_n * inv_sqrt2,            # scale_n
        -mu_n * isd_n * inv_sqrt2,    # bias_n
        isd_a * inv_sqrt2,            # scale_a
        -mu_a * isd_a * inv_sqrt2,    # bias_a
    ], axis=1).astype(np.float32)     # [128, 4]

    half_sqrt_pi = 0.5 * np.sqrt(np.pi)
    c_n = (INV_SQRT_2PI * isd_n * half_sqrt_pi).astype(np.float32)
    c_a = (INV_SQRT_2PI * isd_a * half_sqrt_pi).astype(np.float32)
    wmat = np.zeros((DIM, 512), dtype=np.float32)
    wmat[:, 128] = c_n
    wmat[:, 384] = c_a

    in_maps = []
    for i in range(NCORES):
        lo = i * R
        hi = min(lo + R, N)
        shard_T = np.zeros((DIM, R), dtype=np.float32)
        shard_T[:, :hi - lo] = x[lo:hi].T
        in_maps.append({"xT": shard_T, "consts": consts, "wmat": wmat})

    nc = _get_compiled()
    res = run_bass_kernel_spmd(nc, in_maps, core_ids=list(range(NCORES)))
    LAST_RESULTS = res

    s_n = np.empty(N, dtype=np.float64)
    s_a = np.empty(N, dtype=np.float64)
    for i in range(NCORES):
        lo = i * R
        hi = min(lo + R, N)
        s_n[lo:hi] = res.results[i]["sn_out"].reshape(-1)[:hi - lo]
        s_a[lo:hi] = res.results[i]["sa_out"].reshape(-1)[:hi - lo]

    # exact recurrence p_k = (p_{k-1} + s_k)/dim as truncated causal
    # convolution: p_k = sum_j (1/dim)^(j+1) s_{k-j}; (1/128)^14 ~ 3e-30.
    a = 1.0 / DIM
    pn = np.zeros(N, dtype=np.float64)
    pa = np.zeros(N, dtype=np.float64)
    wgt = a
    for j in range(14):
        if j == 0:
            pn += wgt * s_n
            pa += wgt * s_a
        else:
            pn[j:] += wgt * s_n[:-j]
            pa[j:] += wgt * s_a[:-j]
        wgt *= a
    total = pn + pa
    out = np.empty((N, 2), dtype=np.float32)
    out[:, 0] = (pn / total).astype(np.float32)
    out[:, 1] = (pa / total).astype(np.float32)
    return out
